# revision 13
# baseline (speedup 1.0000x reference)
"""Trainium2 Bass kernel for nn_Block3D (LKP3D dynamic-kernel gen + SKA3D + FFN).

Sharding: 8 cores = batch(2) x D-quarters(4). Each core computes 8 D-slices
with host-prepared circular halos; GroupNorm stats are AllReduce'd across the
4 cores sharing a batch.

Per-core pipeline:
  S1  a1 = relu(bn1(w1 @ x))            PE f32r -> ACT evict into zero-padded,
                                        4x W-shift-replicated buffer (bf16)
  S2  a3 = relu(bn3(w3 @ bn2(dw5(a1)))) dw5 merged with w3*diag(s2) into 125
                                        dense 32x32 matmuls, K-packed 4 taps
  S3  wk = w4 @ a3 + b4; GroupNorm      stats via ACT accum_out + tiny
                                        AllReduce; GN folded to per-partition
                                        affine, applied in-place
  S4  wk -> (h-block, group)-partition layout via DMA transpose
  S5  SKA: 27 shifted DVE mult/adds (bf16), wk broadcast over the 8 channels
      of a group via a stride-0 free dim (no data movement)
  S6+ BN + residual (fp32), FFN (pw1/pw2), final residual, output
"""

import numpy as np

B, C, D, H, W = 2, 64, 32, 32, 32
G, K3 = 8, 27
GN_EPS = 1e-5

_CACHE = {}


# ---------------------------------------------------------------------------
# workarounds for this walrus build (max one sem wait per TPB instruction)
# ---------------------------------------------------------------------------

def _apply_tile_patches():
    from concourse import tile as _tile_mod
    from concourse.vector_clock import ScopedClock, VectorClock

    if getattr(_tile_mod.TileContext, "_drain_patched", False):
        return

    def _patched_drain_and_barrier(self, tick_clock, wait_clock):
        nc = self.nc
        gc = tick_clock.global_clock
        n = len(gc)
        vals = list(gc)
        for i in range(n):
            if vals[i] > 0:
                vec = [0] * n
                vec[i] = vals[i]
                nop_inst = nc.sync.nop(nofuse=True)
                wait_clock.add_sem_waits(
                    nop_inst.ins, ScopedClock({None: VectorClock(vec)})
                )
        nc.sync.drain()
        nc.all_engine_barrier()
        assert self.sems is not None
        popped = nc._tile_sem_poison_stack.pop()
        assert popped is self._sem_poison
        nc.clear_and_free_semaphores(list(self.sems.allocated().values()))
        nc.all_engine_barrier()

    _tile_mod.TileContext._drain_and_barrier = _patched_drain_and_barrier
    _tile_mod.TileContext._drain_patched = True


_WSPLIT = [0]


def _split_waits(nc):
    import concourse.mybir as mybir

    for _name, bbb in list(nc.bb_map.items()):
        bb = bbb.bb if hasattr(bbb, "bb") else bbb
        insts = bb.instructions
        new = []
        changed = False
        for inst in insts:
            si = inst.sync_info
            if si is not None and si.on_wait is not None and len(si.on_wait) > 1:
                waits = list(si.on_wait)
                for w in waits[:-1]:
                    _WSPLIT[0] += 1
                    new.append(
                        mybir.InstNoOp(
                            name=f"wsplit-{_WSPLIT[0]}",
                            engine=inst.engine,
                            sync_info=mybir.SyncInfo(on_wait=[w], on_update=[]),
                        )
                    )
                si.on_wait = [waits[-1]]
                inst.sync_info = si
                changed = True
            new.append(inst)
        if changed:
            bb.instructions[:] = new


# ---------------------------------------------------------------------------
# device program
# ---------------------------------------------------------------------------

def _build_program():
    import concourse.mybir as mybir
    from concourse import bass
    from concourse.tile import TileContext

    _apply_tile_patches()

    F32 = mybir.dt.float32
    F32R = mybir.dt.float32r
    BF16 = mybir.dt.bfloat16
    ALU = mybir.AluOpType
    AF = mybir.ActivationFunctionType

    nc = bass.Bass()

    def din(name, shape, dt=F32):
        return nc.declare_dram_parameter(name, list(shape), dt, isOutput=False)

    xa_d = din("xa", [128, 6, 32, 32])             # (dpair, c) x (6d, h, w)
    xb_d = din("xb", [128, 8, 10, 4, 34], BF16)    # (hb, g) x (c, d', h'', w)
    dmL_d = din("dmL", [32, 2], BF16)
    dmR_d = din("dmR", [32, 2], BF16)
    w1s_d = din("w1s", [128, 32])
    s1v_d = din("s1v", [32, 1])
    b1v_d = din("b1v", [32, 1])
    mst_d = din("mst", [128, 25, 32], BF16)
    mst2_d = din("mst2", [32, 25, 32], BF16)
    s3v_d = din("s3v", [32, 1])
    bias3_d = din("bias3", [32, 1])
    w4lo_d = din("w4lo", [64, 108])
    w4hi_d = din("w4hi", [64, 108])
    b4lo_d = din("b4lo", [108, 1])
    b4hi_d = din("b4hi", [108, 1])
    gglo_d = din("gglo", [108, 1])
    gghi_d = din("gghi", [108, 1])
    gblo_d = din("gblo", [108, 1])
    gbhi_d = din("gbhi", [108, 1])
    sel_d = din("sel", [108, 4])
    rep_d = din("rep", [36, 108])
    bns_d = din("bns", [128, 1])
    bnb_d = din("bnb", [128, 1])
    pw1s_d = din("pw1s", [128, 128])
    pw1sv_d = din("pw1sv", [128, 1])
    pw1bv_d = din("pw1bv", [128, 1])
    pw2s_d = din("pw2s", [128, 64], BF16)
    pw2sv_d = din("pw2sv", [64, 1])
    pw2bv_d = din("pw2bv", [64, 1])

    yout_d = nc.declare_dram_parameter("yout", [64, 8, 32, 32], F32, isOutput=True)

    NTOT = float(K3 * D * H * W)
    OFFS = [(di, hi, wi) for di in (-1, 0, 1) for hi in (-1, 0, 1)
            for wi in (-1, 0, 1)]

    with TileContext(nc) as tc:
        with tc.tile_pool(name="sb", bufs=1) as P, \
             tc.tile_pool(name="ps", bufs=2, space="PSUM") as PS, \
             tc.tile_pool(name="dram", bufs=1, space="DRAM") as PD:

            # persistent inputs / weights
            xb = P.tile([128, 8, 10, 4, 34], BF16, tag="xb")
            xa = P.tile([128, 6, 32, 32], F32, tag="xa")
            w1s = P.tile([128, 32], F32, tag="w1s")
            s1v = P.tile([32, 1], F32, tag="s1v")
            b1v = P.tile([32, 1], F32, tag="b1v")
            dmL = P.tile([32, 2], BF16, tag="dmL")
            dmR = P.tile([32, 2], BF16, tag="dmR")
            mst = P.tile([128, 25, 32], BF16, tag="mst")
            mst2 = P.tile([32, 25, 32], BF16, tag="mst2")
            s3v = P.tile([32, 1], F32, tag="s3v")
            bias3 = P.tile([32, 1], F32, tag="bias3")
            w4h = [P.tile([64, 108], F32, tag=f"w4{h}", name=f"w4{h}") for h in range(2)]
            b4 = [P.tile([108, 1], F32, tag=f"b4{h}", name=f"b4{h}") for h in range(2)]
            gg = [P.tile([108, 1], F32, tag=f"gg{h}", name=f"gg{h}") for h in range(2)]
            gb = [P.tile([108, 1], F32, tag=f"gb{h}", name=f"gb{h}") for h in range(2)]
            sel = P.tile([108, 4], F32, tag="sel")
            rep = P.tile([36, 108], F32, tag="rep")
            bns = P.tile([128, 1], F32, tag="bns")
            bnb = P.tile([128, 1], F32, tag="bnb")
            pw1s = P.tile([128, 128], F32, tag="pw1s")
            pw1sv = P.tile([128, 1], F32, tag="pw1sv")
            pw1bv = P.tile([128, 1], F32, tag="pw1bv")
            pw2s = P.tile([128, 64], BF16, tag="pw2s")
            pw2sv = P.tile([64, 1], F32, tag="pw2sv")
            pw2bv = P.tile([64, 1], F32, tag="pw2bv")

            for t, d in [(xb, xb_d), (xa, xa_d), (w1s, w1s_d), (s1v, s1v_d),
                         (b1v, b1v_d), (dmL, dmL_d), (dmR, dmR_d),
                         (mst, mst_d), (mst2, mst2_d), (s3v, s3v_d),
                         (bias3, bias3_d), (w4h[0], w4lo_d), (w4h[1], w4hi_d),
                         (b4[0], b4lo_d), (b4[1], b4hi_d),
                         (gg[0], gglo_d), (gg[1], gghi_d),
                         (gb[0], gblo_d), (gb[1], gbhi_d),
                         (sel, sel_d), (rep, rep_d), (bns, bns_d), (bnb, bnb_d),
                         (pw1s, pw1s_d), (pw1sv, pw1sv_d), (pw1bv, pw1bv_d),
                         (pw2s, pw2s_d), (pw2sv, pw2sv_d), (pw2bv, pw2bv_d)]:
                nc.sync.dma_start(out=t[:], in_=d[:])

            # ---- S1: a1 into a1rep block 0 (zero-padded, bf16)
            a1rep = P.tile([128, 12, 36, 36], BF16, tag="slot1")
            nc.gpsimd.memset(a1rep[:], 0.0)
            for p in range(2):
                for i in range(12):
                    dd = p * 6 + i // 2
                    hh = i % 2
                    ps = PS.tile([32, 16, 32], F32, tag="mm")
                    nc.tensor.matmul(
                        ps[:],
                        lhsT=w1s[p * 64:(p + 1) * 64, :],
                        rhs=xa[p * 64:(p + 1) * 64, i // 2,
                               hh * 16:(hh + 1) * 16, :],
                        start=True, stop=True)
                    nc.scalar.activation(
                        out=a1rep[0:32, dd, 2 + hh * 16:2 + (hh + 1) * 16, 2:34],
                        in_=ps[:], func=AF.Relu, bias=b1v[:], scale=s1v[:])
            # zero a1 halo slices that fall outside the global D range
            nc.vector.tensor_tensor(
                out=a1rep[0:32, 0:2, :, :], in0=a1rep[0:32, 0:2, :, :],
                in1=dmL[:, :, None, None].to_broadcast((32, 2, 36, 36)),
                op=ALU.mult)
            nc.vector.tensor_tensor(
                out=a1rep[0:32, 10:12, :, :], in0=a1rep[0:32, 10:12, :, :],
                in1=dmR[:, :, None, None].to_broadcast((32, 2, 36, 36)),
                op=ALU.mult)
            for j in range(1, 4):
                nc.sync.dma_start(
                    out=a1rep[j * 32:(j + 1) * 32, :, :, 0:36 - j],
                    in_=a1rep[0:32, :, :, j:36])

            # ---- S2: merged dw5 + w3
            a3t = [P.tile([64, 2, 32, 32], F32, tag=f"slot3{i}", name=f"a3{i}") for i in range(2)]
            for d in range(8):
                for hh in range(2):
                    ps = PS.tile([32, 16, 32], F32, tag="mm")
                    for td in range(5):
                        for th in range(5):
                            t = td * 5 + th
                            hs = hh * 16 + th
                            nc.tensor.matmul(
                                ps[:], lhsT=mst[:, t, :],
                                rhs=a1rep[:, d + td, hs:hs + 16, 0:32],
                                start=(t == 0), stop=False)
                            nc.tensor.matmul(
                                ps[:], lhsT=mst2[:, t, :],
                                rhs=a1rep[0:32, d + td, hs:hs + 16, 4:36],
                                start=False, stop=(t == 24))
                    q = d // 2
                    nc.scalar.activation(
                        out=a3t[q // 2][(q % 2) * 32:(q % 2) * 32 + 32, d % 2,
                                        hh * 16:(hh + 1) * 16, :],
                        in_=ps[:], func=AF.Relu, bias=bias3[:], scale=s3v[:])

            # ---- S3: wk = w4 @ a3 + b4 (bf16), stats via accum_out
            wkA = [P.tile([108, 8, 32, 32], BF16, tag=f"slot2{h}", name=f"wkA{h}") for h in range(2)]
            sums = [P.tile([108, 2, 16], F32, tag=f"sums{h}", name=f"sums{h}") for h in range(2)]
            scr = P.tile([108, 512], F32, tag="scr")
            for h in range(2):
                for q in range(4):
                    for c2 in range(4):
                        idx = q * 4 + c2
                        dd = q * 2 + c2 // 2
                        hh = c2 % 2
                        ps = PS.tile([108, 512], F32, tag="mm")
                        nc.tensor.matmul(
                            ps[:],
                            lhsT=w4h[h][(q % 2) * 32:(q % 2) * 32 + 32, :],
                            rhs=a3t[q // 2][(q % 2) * 32:(q % 2) * 32 + 32, c2 // 2,
                                            hh * 16:(hh + 1) * 16, :],
                            start=True, stop=True)
                        nc.scalar.activation(
                            out=wkA[h][0:108, dd, hh * 16:(hh + 1) * 16, :],
                            in_=ps[:], func=AF.Identity, bias=b4[h][:],
                            accum_out=sums[h][:, 0, idx:idx + 1])
                        nc.scalar.activation(
                            out=scr[:], in_=ps[:], func=AF.Square, bias=b4[h][:],
                            accum_out=sums[h][:, 1, idx:idx + 1])

            # stats reduce -> [8, 2] -> AllReduce over the 4 same-batch cores
            s2t = [P.tile([108, 2], F32, tag=f"s2t{h}", name=f"s2t{h}") for h in range(2)]
            gstats = P.tile([36, 2], F32, tag="gstats")
            nc.vector.memset(gstats[:], 0.0)
            for h in range(2):
                nc.vector.tensor_reduce(
                    out=s2t[h][:], in_=sums[h][:], axis=mybir.AxisListType.X,
                    op=ALU.add)
                ps = PS.tile([4, 2], F32, tag="mmt")
                nc.tensor.matmul(ps[:], lhsT=sel[:], rhs=s2t[h][:],
                                 start=True, stop=True)
                nc.scalar.activation(out=gstats[h * 32:h * 32 + 4, :], in_=ps[:],
                                     func=AF.Copy)
            cin = PD.tile([36, 2], F32)
            cout = PD.tile([36, 2], F32)
            nc.sync.dma_start(out=cin[:], in_=gstats[:])
            nc.gpsimd.collective_compute(
                "AllReduce", ALU.add,
                replica_groups=[[0, 1, 2, 3], [4, 5, 6, 7]],
                ins=[cin[:].opt()], outs=[cout[:].opt()])
            gsum = P.tile([36, 2], F32, tag="gsum")
            nc.sync.dma_start(out=gsum[:], in_=cout[:])

            # mu, rsqrt(var+eps)
            m2 = P.tile([36, 2], F32, tag="m2")
            musq = P.tile([36, 1], F32, tag="musq")
            vs = P.tile([36, 1], F32, tag="vs")
            rv = P.tile([36, 1], F32, tag="rv")
            rs = P.tile([36, 1], F32, tag="rs")
            nc.scalar.mul(m2[:], gsum[:], 1.0 / NTOT)
            nc.scalar.activation(out=musq[:], in_=m2[:, 0:1], func=AF.Square)
            nc.vector.tensor_tensor(out=vs[:], in0=m2[:, 1:2], in1=musq[:],
                                    op=ALU.subtract)
            nc.vector.tensor_scalar_add(vs[:], vs[:], GN_EPS)
            nc.vector.reciprocal(rv[:], vs[:])
            nc.scalar.activation(out=rs[:], in_=rv[:], func=AF.Sqrt)

            # per-partition GN affine: alpha = r*gamma, beta = gn_b - mu*alpha
            for h in range(2):
                psr = PS.tile([108, 1], F32, tag="mmt")
                nc.tensor.matmul(psr[:], lhsT=rep[h * 32:h * 32 + 4, :], rhs=rs[h * 32:h * 32 + 4, :],
                                 start=True, stop=True)
                rb = P.tile([108, 1], F32, tag=f"rb{h}")
                nc.scalar.activation(out=rb[:], in_=psr[:], func=AF.Copy)
                psm = PS.tile([108, 1], F32, tag="mmt")
                nc.tensor.matmul(psm[:], lhsT=rep[h * 32:h * 32 + 4, :],
                                 rhs=m2[h * 32:h * 32 + 4, 0:1],
                                 start=True, stop=True)
                mb = P.tile([108, 1], F32, tag=f"mb{h}")
                nc.scalar.activation(out=mb[:], in_=psm[:], func=AF.Copy)
                alpha = P.tile([108, 1], F32, tag=f"al{h}")
                beta = P.tile([108, 1], F32, tag=f"be{h}")
                nc.vector.tensor_tensor(out=alpha[:], in0=rb[:], in1=gg[h][:],
                                        op=ALU.mult)
                nc.vector.tensor_tensor(out=beta[:], in0=mb[:], in1=alpha[:],
                                        op=ALU.mult)
                nc.vector.tensor_tensor(out=beta[:], in0=gb[h][:], in1=beta[:],
                                        op=ALU.subtract)
                # GN apply in place on wkA[h]
                nc.scalar.activation(out=wkA[h][:], in_=wkA[h][:],
                                     func=AF.Identity, bias=beta[:],
                                     scale=alpha[:])

            # ---- S4: transpose wk into (hb, g) partition layout
            wkB = P.tile([128, 27, 8, 2, 32], BF16, tag="slot1")
            for hb in range(16):
                for h in range(2):
                    nc.sync.dma_start(
                        out=wkB[hb * 8 + h * 4:hb * 8 + h * 4 + 4, :, :, :, :],
                        in_=wkA[h][0:108, :, 2 * hb:2 * hb + 2, :])

            # ---- S5: SKA
            accB = P.tile([128, 8, 8, 2, 32], BF16, tag="slot4")
            prodB = P.tile([128, 8, 8, 2, 32], BF16, tag="slot5")
            for k, (di, hi, wi) in enumerate(OFFS):
                for hp in range(2):
                    xsl = xb[:, :, 1 + di:9 + di, 1 + hi + hp, 1 + wi:33 + wi]
                    wsl = wkB[:, k:k + 1, :, hp, :].to_broadcast((128, 8, 8, 32))
                    if k == 0:
                        nc.vector.tensor_tensor(out=accB[:, :, :, hp, :],
                                                in0=xsl, in1=wsl, op=ALU.mult)
                    else:
                        nc.vector.tensor_tensor(out=prodB[:, :, :, hp, :],
                                                in0=xsl, in1=wsl, op=ALU.mult)
                        nc.vector.tensor_tensor(out=accB[:, :, :, hp, :],
                                                in0=accB[:, :, :, hp, :],
                                                in1=prodB[:, :, :, hp, :],
                                                op=ALU.add)

            # convert to f32 for the transpose back
            accF = P.tile([128, 8, 8, 2, 32], F32, tag="slot5")
            nc.vector.tensor_copy(accF[:], accB[:])

            # ---- S7: transpose sk back to channel-partition layout
            skA = P.tile([128, 16, 4, 2, 32], F32, tag="slot30")
            for hb in range(16):
                for dh in range(2):
                    nc.sync.dma_start(
                        out=skA[dh * 64:(dh + 1) * 64, hb, :, :, :],
                        in_=accF[hb * 8:(hb + 1) * 8, :, dh * 4:(dh + 1) * 4, :, :])

            # ---- S6': y = bn_s*sk + bn_b + x   (fp32)
            yA = P.tile([128, 16, 4, 2, 32], F32, tag="yA")
            for dh in range(2):
                xs_all = xa[0:64, 2:6, :, :] if dh == 0 else xa[64:128, 0:4, :, :]
                for hb in range(16):
                    nc.vector.scalar_tensor_tensor(
                        out=yA[dh * 64:(dh + 1) * 64, hb, :, :, :],
                        in0=skA[dh * 64:(dh + 1) * 64, hb, :, :, :],
                        scalar=bns[dh * 64:(dh + 1) * 64, :],
                        in1=xs_all[:, :, 2 * hb:2 * hb + 2, :],
                        op0=ALU.mult, op1=ALU.add)
                nc.vector.tensor_scalar_add(
                    yA[dh * 64:(dh + 1) * 64, :, :, :, :],
                    yA[dh * 64:(dh + 1) * 64, :, :, :, :],
                    bnb[dh * 64:(dh + 1) * 64, :])

            # ---- S8: FFN
            f1 = P.tile([128, 2, 16, 4, 2, 32], BF16, tag="slot20")
            for dh in range(2):
                for ch in range(8):
                    ps = PS.tile([128, 512], F32, tag="mm")
                    nc.tensor.matmul(
                        ps[:],
                        lhsT=pw1s[dh * 64:(dh + 1) * 64, :],
                        rhs=yA[dh * 64:(dh + 1) * 64, ch * 2:(ch + 1) * 2,
                               :, :, :],
                        start=True, stop=True)
                    nc.scalar.activation(
                        out=f1[:, dh, ch * 2:(ch + 1) * 2, :, :, :],
                        in_=ps[:], func=AF.Relu, bias=pw1bv[:], scale=pw1sv[:])
            f2t = P.tile([128, 16, 4, 2, 32], F32, tag="slot21")
            for dh in range(2):
                for ch in range(8):
                    ps = PS.tile([64, 512], F32, tag="mm")
                    nc.tensor.matmul(
                        ps[:], lhsT=pw2s[:],
                        rhs=f1[:, dh, ch * 2:(ch + 1) * 2, :, :, :],
                        start=True, stop=True)
                    nc.scalar.activation(
                        out=f2t[dh * 64:(dh + 1) * 64, ch * 2:(ch + 1) * 2,
                                :, :, :],
                        in_=ps[:], func=AF.Identity, bias=pw2bv[:],
                        scale=pw2sv[:])

            # ---- S9: out = y + f, write back
            outT = P.tile([128, 16, 4, 2, 32], F32, tag="slot4")
            nc.vector.tensor_tensor(out=outT[:], in0=yA[:], in1=f2t[:],
                                    op=ALU.add)
            for dh in range(2):
                for d in range(4):
                    nc.sync.dma_start(
                        out=yout_d[:, dh * 4 + d, :, :],
                        in_=outT[dh * 64:(dh + 1) * 64, :, d, :, :])

    _split_waits(nc)
    return nc


# ---------------------------------------------------------------------------
# host side
# ---------------------------------------------------------------------------

def _prep_inputs(inputs):
    import ml_dtypes

    x = np.asarray(inputs["x"], np.float32)
    w1 = np.asarray(inputs["w1"], np.float32)
    s1 = np.asarray(inputs["s1"], np.float32)
    b1 = np.asarray(inputs["b1"], np.float32)
    w2 = np.asarray(inputs["w2"], np.float32)
    s2 = np.asarray(inputs["s2"], np.float32)
    b2 = np.asarray(inputs["b2"], np.float32)
    w3 = np.asarray(inputs["w3"], np.float32)
    s3 = np.asarray(inputs["s3"], np.float32)
    b3 = np.asarray(inputs["b3"], np.float32)
    w4 = np.asarray(inputs["w4"], np.float32)
    b4 = np.asarray(inputs["b4"], np.float32)
    gn_g = np.asarray(inputs["gn_g"], np.float32)
    gn_b = np.asarray(inputs["gn_b"], np.float32)
    bn_s = np.asarray(inputs["bn_s"], np.float32)
    bn_b = np.asarray(inputs["bn_b"], np.float32)
    pw1_w = np.asarray(inputs["pw1_w"], np.float32)
    pw1_s = np.asarray(inputs["pw1_s"], np.float32)
    pw1_b = np.asarray(inputs["pw1_b"], np.float32)
    pw2_w = np.asarray(inputs["pw2_w"], np.float32)
    pw2_s = np.asarray(inputs["pw2_s"], np.float32)
    pw2_b = np.asarray(inputs["pw2_b"], np.float32)

    bf16 = ml_dtypes.bfloat16

    # shared weight tensors
    shared = {}
    shared["w1s"] = np.ascontiguousarray(np.tile(w1.T, (2, 1)))     # [128, 32]
    shared["s1v"] = s1.reshape(32, 1)
    shared["b1v"] = b1.reshape(32, 1)
    base = (w3 * (s2[None, :])).T                                   # [c, o]
    full = np.einsum('co,cxyz->xyzco', base, w2[:, 0])              # [5,5,5,c,o]
    mst = np.zeros((25, 128, 32), np.float32)
    mst2 = np.zeros((25, 32, 32), np.float32)
    for td in range(5):
        for th in range(5):
            t = td * 5 + th
            mst[t] = full[td, th, 0:4].reshape(128, 32)
            mst2[t] = full[td, th, 4]
    shared["mst"] = mst.transpose(1, 0, 2).astype(bf16)             # [128,25,32]
    shared["mst2"] = mst2.transpose(1, 0, 2).astype(bf16)           # [32,25,32]
    shared["s3v"] = s3.reshape(32, 1)
    shared["bias3"] = (s3 * (w3 @ b2) + b3).reshape(32, 1)
    w4T = np.ascontiguousarray(w4.T)                                # [32, 216]
    shared["w4lo"] = np.ascontiguousarray(np.tile(w4T[:, :108], (2, 1)))
    shared["w4hi"] = np.ascontiguousarray(np.tile(w4T[:, 108:], (2, 1)))
    shared["b4lo"] = b4[:108].reshape(108, 1)
    shared["b4hi"] = b4[108:].reshape(108, 1)
    shared["gglo"] = gn_g[:108].reshape(108, 1)
    shared["gghi"] = gn_g[108:].reshape(108, 1)
    shared["gblo"] = gn_b[:108].reshape(108, 1)
    shared["gbhi"] = gn_b[108:].reshape(108, 1)
    selm = np.zeros((108, 4), np.float32)
    for g in range(4):
        selm[g * 27:(g + 1) * 27, g] = 1.0
    shared["sel"] = selm
    repm = np.zeros((36, 108), np.float32)
    repm[0:4] = selm.T
    repm[32:36] = selm.T
    shared["rep"] = repm
    shared["bns"] = np.tile(bn_s, 2).reshape(128, 1)
    shared["bnb"] = np.tile(bn_b, 2).reshape(128, 1)
    shared["pw1s"] = np.ascontiguousarray(np.tile(pw1_w.T, (2, 1))) # [128, 128]
    shared["pw1sv"] = pw1_s.reshape(128, 1)
    shared["pw1bv"] = pw1_b.reshape(128, 1)
    shared["pw2s"] = np.ascontiguousarray(pw2_w.T).astype(bf16)     # [128, 64]
    shared["pw2sv"] = pw2_s.reshape(64, 1)
    shared["pw2bv"] = pw2_b.reshape(64, 1)

    in_maps = []
    for core in range(8):
        b, dq = core // 4, core % 4
        d0 = dq * 8
        m = dict(shared)

        # xa: [(2 dpair, 64 c), 6, 32, 32], d slices d0 + [-2..10), wrapped
        d_idx = (d0 + np.arange(-2, 10)) % D
        xd = x[b][:, d_idx]                            # [64, 12, 32, 32]
        xa = np.empty((128, 6, 32, 32), np.float32)
        xa[0:64] = xd[:, 0:6]
        xa[64:128] = xd[:, 6:12]
        m["xa"] = xa

        # xb: [(16 hb, 8 g), 8 c, 10 d', 4 h'', 34 w]
        d_idx2 = (d0 + np.arange(-1, 9)) % D
        xd2 = x[b][:, d_idx2]                          # [64, 10, 32, 32]
        xw = np.concatenate([xd2[..., -1:], xd2, xd2[..., :1]], axis=-1)
        h_idx = (2 * np.arange(16)[:, None] + np.arange(-1, 3)[None, :]) % H
        xh = xw[:, :, h_idx]                           # [64, 10, 16, 4, 34]
        arr = xh.reshape(8, 8, 10, 16, 4, 34)
        m["xb"] = np.ascontiguousarray(
            arr.transpose(3, 0, 1, 2, 4, 5).reshape(128, 8, 10, 4, 34)
        ).astype(bf16)

        m["dmL"] = np.full((32, 2), 0.0 if dq == 0 else 1.0, bf16)
        m["dmR"] = np.full((32, 2), 0.0 if dq == 3 else 1.0, bf16)
        in_maps.append(m)
    return in_maps


def _ensure_ntff_hook():
    import sys, types
    try:
        from antenv.axon_hooks import get_axon_ntff_profile_hook  # noqa
        return
    except ImportError:
        pass
    mod = types.ModuleType("antenv.axon_hooks")
    _h = [None]
    mod.set_axon_ntff_profile_hook = lambda h: _h.__setitem__(0, h)
    mod.get_axon_ntff_profile_hook = lambda: _h[0]
    sys.modules["antenv.axon_hooks"] = mod
    import antenv
    antenv.axon_hooks = mod
    try:
        from trn_agent_boot.trn_boot import _ntff_profile_via_ctypes
        mod.set_axon_ntff_profile_hook(
            _ntff_profile_via_ctypes("/opt/axon/libaxon_pjrt.so"))
    except Exception:
        pass


def kernel(**inputs):
    import os
    from concourse.bass_utils import run_bass_kernel_spmd

    if "nc" not in _CACHE:
        _CACHE["nc"] = _build_program()
    nc = _CACHE["nc"]

    in_maps = _prep_inputs(inputs)
    trace = bool(os.environ.get("KERNEL_TRACE"))
    if trace:
        _ensure_ntff_hook()
    res = run_bass_kernel_spmd(nc, in_maps, list(range(8)), trace=trace)
    globals()["LAST_EXEC_NS"] = res.exec_time_ns
    if trace and res.profile_json is not None:
        globals()["LAST_PROFILE"] = res.profile_json

    out = np.empty((B, C, D, H, W), np.float32)
    for core in range(8):
        b, dq = core // 4, core % 4
        out[b, :, dq * 8:(dq + 1) * 8] = res.results[core]["yout"]
    return out


# revision 16
# speedup vs baseline: 1.0211x; 1.0211x over previous
"""Trainium2 Bass kernel for nn_Block3D (LKP3D dynamic-kernel gen + SKA3D + FFN).

Sharding: 8 cores = batch(2) x D-quarters(4). Each core computes 8 D-slices
with host-prepared circular halos; GroupNorm stats are AllReduce'd across the
4 cores sharing a batch.

Per-core pipeline:
  S1  a1 = relu(bn1(w1 @ x))            PE f32r -> ACT evict into zero-padded,
                                        4x W-shift-replicated buffer (bf16)
  S2  a3 = relu(bn3(w3 @ bn2(dw5(a1)))) dw5 merged with w3*diag(s2) into 125
                                        dense 32x32 matmuls, K-packed 4 taps
  S3  wk = w4 @ a3 + b4; GroupNorm      stats via ACT accum_out + tiny
                                        AllReduce; GN folded to per-partition
                                        affine, applied in-place
  S4  wk -> (h-block, group)-partition layout via DMA transpose
  S5  SKA: 27 shifted DVE mult/adds (bf16), wk broadcast over the 8 channels
      of a group via a stride-0 free dim (no data movement)
  S6+ BN + residual (fp32), FFN (pw1/pw2), final residual, output
"""

import numpy as np

B, C, D, H, W = 2, 64, 32, 32, 32
G, K3 = 8, 27
GN_EPS = 1e-5

_CACHE = {}


# ---------------------------------------------------------------------------
# workarounds for this walrus build (max one sem wait per TPB instruction)
# ---------------------------------------------------------------------------

def _apply_tile_patches():
    from concourse import tile as _tile_mod
    from concourse.vector_clock import ScopedClock, VectorClock

    if getattr(_tile_mod.TileContext, "_drain_patched", False):
        return

    def _patched_drain_and_barrier(self, tick_clock, wait_clock):
        nc = self.nc
        gc = tick_clock.global_clock
        n = len(gc)
        vals = list(gc)
        for i in range(n):
            if vals[i] > 0:
                vec = [0] * n
                vec[i] = vals[i]
                nop_inst = nc.sync.nop(nofuse=True)
                wait_clock.add_sem_waits(
                    nop_inst.ins, ScopedClock({None: VectorClock(vec)})
                )
        nc.sync.drain()
        nc.all_engine_barrier()
        assert self.sems is not None
        popped = nc._tile_sem_poison_stack.pop()
        assert popped is self._sem_poison
        nc.clear_and_free_semaphores(list(self.sems.allocated().values()))
        nc.all_engine_barrier()

    _tile_mod.TileContext._drain_and_barrier = _patched_drain_and_barrier
    _tile_mod.TileContext._drain_patched = True


_WSPLIT = [0]


def _split_waits(nc):
    import concourse.mybir as mybir

    for _name, bbb in list(nc.bb_map.items()):
        bb = bbb.bb if hasattr(bbb, "bb") else bbb
        insts = bb.instructions
        new = []
        changed = False
        for inst in insts:
            si = inst.sync_info
            if si is not None and si.on_wait is not None and len(si.on_wait) > 1:
                waits = list(si.on_wait)
                for w in waits[:-1]:
                    _WSPLIT[0] += 1
                    new.append(
                        mybir.InstNoOp(
                            name=f"wsplit-{_WSPLIT[0]}",
                            engine=inst.engine,
                            sync_info=mybir.SyncInfo(on_wait=[w], on_update=[]),
                        )
                    )
                si.on_wait = [waits[-1]]
                inst.sync_info = si
                changed = True
            new.append(inst)
        if changed:
            bb.instructions[:] = new


# ---------------------------------------------------------------------------
# device program
# ---------------------------------------------------------------------------

def _build_program():
    import concourse.mybir as mybir
    from concourse import bass
    from concourse.tile import TileContext

    _apply_tile_patches()

    F32 = mybir.dt.float32
    F32R = mybir.dt.float32r
    BF16 = mybir.dt.bfloat16
    ALU = mybir.AluOpType
    AF = mybir.ActivationFunctionType

    nc = bass.Bass()

    def din(name, shape, dt=F32):
        return nc.declare_dram_parameter(name, list(shape), dt, isOutput=False)

    xa_d = din("xa", [128, 6, 32, 32])             # (dpair, c) x (6d, h, w)
    xb_d = din("xb", [128, 8, 10, 4, 34], BF16)    # (hb, g) x (c, d', h'', w)
    dmL_d = din("dmL", [128, 2], BF16)
    dmR_d = din("dmR", [128, 2], BF16)
    w1s_d = din("w1s", [128, 32])
    s1v_d = din("s1v", [32, 1])
    b1v_d = din("b1v", [32, 1])
    mst_d = din("mst", [128, 25, 32], BF16)
    mst2_d = din("mst2", [32, 25, 32], BF16)
    s3v_d = din("s3v", [32, 1])
    bias3_d = din("bias3", [32, 1])
    w4lo_d = din("w4lo", [64, 108])
    w4hi_d = din("w4hi", [64, 108])
    b4lo_d = din("b4lo", [108, 1])
    b4hi_d = din("b4hi", [108, 1])
    gglo_d = din("gglo", [108, 1])
    gghi_d = din("gghi", [108, 1])
    gblo_d = din("gblo", [108, 1])
    gbhi_d = din("gbhi", [108, 1])
    sel_d = din("sel", [108, 4])
    rep_d = din("rep", [36, 108])
    bns_d = din("bns", [128, 1])
    bnb_d = din("bnb", [128, 1])
    pw1s_d = din("pw1s", [128, 128])
    pw1sv_d = din("pw1sv", [128, 1])
    pw1bv_d = din("pw1bv", [128, 1])
    pw2s_d = din("pw2s", [128, 64], BF16)
    pw2sv_d = din("pw2sv", [64, 1])
    pw2bv_d = din("pw2bv", [64, 1])

    yout_d = nc.declare_dram_parameter("yout", [64, 8, 32, 32], F32, isOutput=True)

    NTOT = float(K3 * D * H * W)
    OFFS = [(di, hi, wi) for di in (-1, 0, 1) for hi in (-1, 0, 1)
            for wi in (-1, 0, 1)]

    with TileContext(nc) as tc:
        with tc.tile_pool(name="sb", bufs=1) as P, \
             tc.tile_pool(name="ps", bufs=2, space="PSUM") as PS, \
             tc.tile_pool(name="dram", bufs=1, space="DRAM") as PD:

            # persistent inputs / weights
            xb = P.tile([128, 8, 10, 4, 34], BF16, tag="xb")
            xa = P.tile([128, 6, 32, 32], F32, tag="xa")
            w1s = P.tile([128, 32], F32, tag="w1s")
            s1v = P.tile([32, 1], F32, tag="s1v")
            b1v = P.tile([32, 1], F32, tag="b1v")
            dmL = P.tile([128, 2], BF16, tag="dmL")
            dmR = P.tile([128, 2], BF16, tag="dmR")
            mst = P.tile([128, 25, 32], BF16, tag="mst")
            mst2 = P.tile([32, 25, 32], BF16, tag="mst2")
            s3v = P.tile([32, 1], F32, tag="s3v")
            bias3 = P.tile([32, 1], F32, tag="bias3")
            w4h = [P.tile([64, 108], F32, tag=f"w4{h}", name=f"w4{h}") for h in range(2)]
            b4 = [P.tile([108, 1], F32, tag=f"b4{h}", name=f"b4{h}") for h in range(2)]
            gg = [P.tile([108, 1], F32, tag=f"gg{h}", name=f"gg{h}") for h in range(2)]
            gb = [P.tile([108, 1], F32, tag=f"gb{h}", name=f"gb{h}") for h in range(2)]
            sel = P.tile([108, 4], F32, tag="sel")
            rep = P.tile([36, 108], F32, tag="rep")
            bns = P.tile([128, 1], F32, tag="bns")
            bnb = P.tile([128, 1], F32, tag="bnb")
            pw1s = P.tile([128, 128], F32, tag="pw1s")
            pw1sv = P.tile([128, 1], F32, tag="pw1sv")
            pw1bv = P.tile([128, 1], F32, tag="pw1bv")
            pw2s = P.tile([128, 64], BF16, tag="pw2s")
            pw2sv = P.tile([64, 1], F32, tag="pw2sv")
            pw2bv = P.tile([64, 1], F32, tag="pw2bv")

            for t, d in [(xb, xb_d), (xa, xa_d), (w1s, w1s_d), (s1v, s1v_d),
                         (b1v, b1v_d), (dmL, dmL_d), (dmR, dmR_d),
                         (mst, mst_d), (mst2, mst2_d), (s3v, s3v_d),
                         (bias3, bias3_d), (w4h[0], w4lo_d), (w4h[1], w4hi_d),
                         (b4[0], b4lo_d), (b4[1], b4hi_d),
                         (gg[0], gglo_d), (gg[1], gghi_d),
                         (gb[0], gblo_d), (gb[1], gbhi_d),
                         (sel, sel_d), (rep, rep_d), (bns, bns_d), (bnb, bnb_d),
                         (pw1s, pw1s_d), (pw1sv, pw1sv_d), (pw1bv, pw1bv_d),
                         (pw2s, pw2s_d), (pw2sv, pw2sv_d), (pw2bv, pw2bv_d)]:
                nc.sync.dma_start(out=t[:], in_=d[:])

            # ---- S1: a1 into a1rep block 0 (zero-padded, bf16)
            a1rep = P.tile([128, 12, 36, 32], BF16, tag="slot1")
            a1rep5 = P.tile([32, 12, 36, 32], BF16, tag="slot20", name="a1rep5")
            nc.gpsimd.memset(a1rep[:], 0.0)
            nc.gpsimd.memset(a1rep5[:], 0.0)
            for p in range(2):
                for i in range(12):
                    dd = p * 6 + i // 2
                    hh = i % 2
                    ps = PS.tile([32, 16, 32], F32, tag="mm")
                    nc.tensor.matmul(
                        ps[:],
                        lhsT=w1s[p * 64:(p + 1) * 64, :],
                        rhs=xa[p * 64:(p + 1) * 64, i // 2,
                               hh * 16:(hh + 1) * 16, :],
                        start=True, stop=True)
                    nc.scalar.activation(
                        out=a1rep[64:96, dd, 2 + hh * 16:2 + (hh + 1) * 16, 0:32],
                        in_=ps[:], func=AF.Relu, bias=b1v[:], scale=s1v[:])
            # zero a1 halo slices that fall outside the global D range
            nc.vector.tensor_tensor(
                out=a1rep[64:96, 0:2, :, :], in0=a1rep[64:96, 0:2, :, :],
                in1=dmL[64:96, :, None, None].to_broadcast((32, 2, 36, 32)),
                op=ALU.mult)
            nc.vector.tensor_tensor(
                out=a1rep[64:96, 10:12, :, :], in0=a1rep[64:96, 10:12, :, :],
                in1=dmR[64:96, :, None, None].to_broadcast((32, 2, 36, 32)),
                op=ALU.mult)
            # block j holds padded-w window [j, j+32); data lives at wp in [2,34)
            for j in (0, 1, 3):
                s0, s1_ = max(0, j - 2), min(32, j + 30)
                nc.sync.dma_start(
                    out=a1rep[j * 32:(j + 1) * 32, :, :, 2 - j + s0:2 - j + s1_],
                    in_=a1rep[64:96, :, :, s0:s1_])
            nc.sync.dma_start(
                out=a1rep5[:, :, :, 0:30],
                in_=a1rep[64:96, :, :, 2:32])

            # ---- S2: merged dw5 + w3
            a3t = [P.tile([64, 2, 32, 32], F32, tag=f"slot3{i}", name=f"a3{i}") for i in range(2)]
            for d in range(8):
                for hh in range(2):
                    ps = PS.tile([32, 16, 32], F32, tag="mm")
                    for td in range(5):
                        for th in range(5):
                            t = td * 5 + th
                            hs = hh * 16 + th
                            nc.tensor.matmul(
                                ps[:], lhsT=mst[:, t, :],
                                rhs=a1rep[:, d + td, hs:hs + 16, :],
                                start=(t == 0), stop=False)
                            nc.tensor.matmul(
                                ps[:], lhsT=mst2[:, t, :],
                                rhs=a1rep5[:, d + td, hs:hs + 16, :],
                                start=False, stop=(t == 24))
                    q = d // 2
                    nc.scalar.activation(
                        out=a3t[q // 2][(q % 2) * 32:(q % 2) * 32 + 32, d % 2,
                                        hh * 16:(hh + 1) * 16, :],
                        in_=ps[:], func=AF.Relu, bias=bias3[:], scale=s3v[:])

            # ---- S3: wk = w4 @ a3 + b4 (bf16), stats via accum_out
            wkA = [P.tile([108, 8, 32, 32], BF16, tag=f"slot2{h}", name=f"wkA{h}") for h in range(2)]
            sums = [P.tile([108, 2, 16], F32, tag=f"sums{h}", name=f"sums{h}") for h in range(2)]
            scr = P.tile([108, 512], F32, tag="scr")
            for h in range(2):
                for q in range(4):
                    for c2 in range(4):
                        idx = q * 4 + c2
                        dd = q * 2 + c2 // 2
                        hh = c2 % 2
                        ps = PS.tile([108, 512], F32, tag="mm")
                        nc.tensor.matmul(
                            ps[:],
                            lhsT=w4h[h][(q % 2) * 32:(q % 2) * 32 + 32, :],
                            rhs=a3t[q // 2][(q % 2) * 32:(q % 2) * 32 + 32, c2 // 2,
                                            hh * 16:(hh + 1) * 16, :],
                            start=True, stop=True)
                        nc.scalar.activation(
                            out=wkA[h][0:108, dd, hh * 16:(hh + 1) * 16, :],
                            in_=ps[:], func=AF.Identity, bias=b4[h][:],
                            accum_out=sums[h][:, 0, idx:idx + 1])
                        nc.scalar.activation(
                            out=scr[:], in_=ps[:], func=AF.Square, bias=b4[h][:],
                            accum_out=sums[h][:, 1, idx:idx + 1])

            # stats reduce -> [8, 2] -> AllReduce over the 4 same-batch cores
            s2t = [P.tile([108, 2], F32, tag=f"s2t{h}", name=f"s2t{h}") for h in range(2)]
            gstats = P.tile([36, 2], F32, tag="gstats")
            nc.vector.memset(gstats[:], 0.0)
            for h in range(2):
                nc.vector.tensor_reduce(
                    out=s2t[h][:], in_=sums[h][:], axis=mybir.AxisListType.X,
                    op=ALU.add)
                ps = PS.tile([4, 2], F32, tag="mmt")
                nc.tensor.matmul(ps[:], lhsT=sel[:], rhs=s2t[h][:],
                                 start=True, stop=True)
                nc.scalar.activation(out=gstats[h * 32:h * 32 + 4, :], in_=ps[:],
                                     func=AF.Copy)
            cin = PD.tile([36, 2], F32)
            cout = PD.tile([36, 2], F32)
            nc.sync.dma_start(out=cin[:], in_=gstats[:])
            nc.gpsimd.collective_compute(
                "AllReduce", ALU.add,
                replica_groups=[[0, 1, 2, 3], [4, 5, 6, 7]],
                ins=[cin[:].opt()], outs=[cout[:].opt()])
            gsum = P.tile([36, 2], F32, tag="gsum")
            nc.sync.dma_start(out=gsum[:], in_=cout[:])

            # mu, rsqrt(var+eps)
            m2 = P.tile([36, 2], F32, tag="m2")
            musq = P.tile([36, 1], F32, tag="musq")
            vs = P.tile([36, 1], F32, tag="vs")
            rv = P.tile([36, 1], F32, tag="rv")
            rs = P.tile([36, 1], F32, tag="rs")
            nc.scalar.mul(m2[:], gsum[:], 1.0 / NTOT)
            nc.scalar.activation(out=musq[:], in_=m2[:, 0:1], func=AF.Square)
            nc.vector.tensor_tensor(out=vs[:], in0=m2[:, 1:2], in1=musq[:],
                                    op=ALU.subtract)
            nc.vector.tensor_scalar_add(vs[:], vs[:], GN_EPS)
            nc.vector.reciprocal(rv[:], vs[:])
            nc.scalar.activation(out=rs[:], in_=rv[:], func=AF.Sqrt)

            # per-partition GN affine: alpha = r*gamma, beta = gn_b - mu*alpha
            alphas, betas = [], []
            for h in range(2):
                psr = PS.tile([108, 1], F32, tag="mmt")
                nc.tensor.matmul(psr[:], lhsT=rep[h * 32:h * 32 + 4, :], rhs=rs[h * 32:h * 32 + 4, :],
                                 start=True, stop=True)
                rb = P.tile([108, 1], F32, tag=f"rb{h}")
                nc.scalar.activation(out=rb[:], in_=psr[:], func=AF.Copy)
                psm = PS.tile([108, 1], F32, tag="mmt")
                nc.tensor.matmul(psm[:], lhsT=rep[h * 32:h * 32 + 4, :],
                                 rhs=m2[h * 32:h * 32 + 4, 0:1],
                                 start=True, stop=True)
                mb = P.tile([108, 1], F32, tag=f"mb{h}")
                nc.scalar.activation(out=mb[:], in_=psm[:], func=AF.Copy)
                alpha = P.tile([108, 1], F32, tag=f"al{h}")
                beta = P.tile([108, 1], F32, tag=f"be{h}")
                nc.vector.tensor_tensor(out=alpha[:], in0=rb[:], in1=gg[h][:],
                                        op=ALU.mult)
                nc.vector.tensor_tensor(out=beta[:], in0=mb[:], in1=alpha[:],
                                        op=ALU.mult)
                nc.vector.tensor_tensor(out=beta[:], in0=gb[h][:], in1=beta[:],
                                        op=ALU.subtract)
                alphab = P.tile([108, 1], BF16, tag=f"alb{h}", name=f"alb{h}")
                betab = P.tile([108, 1], BF16, tag=f"beb{h}", name=f"beb{h}")
                nc.vector.tensor_copy(alphab[:], alpha[:])
                nc.vector.tensor_copy(betab[:], beta[:])
                alphas.append(alphab)
                betas.append(betab)

            # ---- S4: transpose RAW wk into (hb, g) partition layout (overlaps
            # with the GN-stats collective), then apply the GN affine in B.
            wkB = P.tile([128, 27, 8, 2, 32], BF16, tag="slot1")
            for hb in range(16):
                for h in range(2):
                    nc.sync.dma_start(
                        out=wkB[hb * 8 + h * 4:hb * 8 + h * 4 + 4, :, :, :, :],
                        in_=wkA[h][0:108, :, 2 * hb:2 * hb + 2, :])
            # distribute alpha/beta to (hb, g) partitions: ab8 rows (g) x (k | k)
            ab8 = P.tile([8, 54], BF16, tag="ab8")
            for h in range(2):
                nc.sync.dma_start(out=ab8[h * 4:(h + 1) * 4, 0:27],
                                  in_=alphas[h][:])
                nc.sync.dma_start(out=ab8[h * 4:(h + 1) * 4, 27:54],
                                  in_=betas[h][:])
            abB = P.tile([128, 54], BF16, tag="abB")
            for hb in range(16):
                nc.sync.dma_start(out=abB[hb * 8:(hb + 1) * 8, :], in_=ab8[:])
            nc.vector.tensor_tensor(
                out=wkB[:], in0=wkB[:],
                in1=abB[:, 0:27, None].to_broadcast((128, 27, 512)),
                op=ALU.mult)
            nc.vector.tensor_tensor(
                out=wkB[:], in0=wkB[:],
                in1=abB[:, 27:54, None].to_broadcast((128, 27, 512)),
                op=ALU.add)

            # ---- S5: SKA
            accB = P.tile([128, 8, 8, 2, 32], BF16, tag="slot4")
            prodB = P.tile([128, 8, 8, 2, 32], BF16, tag="slot5")
            for k, (di, hi, wi) in enumerate(OFFS):
                for hp in range(2):
                    xsl = xb[:, :, 1 + di:9 + di, 1 + hi + hp, 1 + wi:33 + wi]
                    wsl = wkB[:, k:k + 1, :, hp, :].to_broadcast((128, 8, 8, 32))
                    if k == 0:
                        nc.vector.tensor_tensor(out=accB[:, :, :, hp, :],
                                                in0=xsl, in1=wsl, op=ALU.mult)
                    else:
                        nc.vector.tensor_tensor(out=prodB[:, :, :, hp, :],
                                                in0=xsl, in1=wsl, op=ALU.mult)
                        nc.vector.tensor_tensor(out=accB[:, :, :, hp, :],
                                                in0=accB[:, :, :, hp, :],
                                                in1=prodB[:, :, :, hp, :],
                                                op=ALU.add)

            # convert to f32 for the transpose back
            accF = P.tile([128, 8, 8, 2, 32], F32, tag="slot5")
            nc.vector.tensor_copy(accF[:], accB[:])

            # ---- S7: transpose sk back to channel-partition layout
            skA = P.tile([128, 16, 4, 2, 32], F32, tag="slot30")
            for hb in range(16):
                for dh in range(2):
                    nc.sync.dma_start(
                        out=skA[dh * 64:(dh + 1) * 64, hb, :, :, :],
                        in_=accF[hb * 8:(hb + 1) * 8, :, dh * 4:(dh + 1) * 4, :, :])

            # ---- S6': y = bn_s*sk + bn_b + x   (fp32)
            yA = P.tile([128, 16, 4, 2, 32], F32, tag="yA")
            for dh in range(2):
                xs_all = xa[0:64, 2:6, :, :] if dh == 0 else xa[64:128, 0:4, :, :]
                for hb in range(16):
                    nc.vector.scalar_tensor_tensor(
                        out=yA[dh * 64:(dh + 1) * 64, hb, :, :, :],
                        in0=skA[dh * 64:(dh + 1) * 64, hb, :, :, :],
                        scalar=bns[dh * 64:(dh + 1) * 64, :],
                        in1=xs_all[:, :, 2 * hb:2 * hb + 2, :],
                        op0=ALU.mult, op1=ALU.add)
                nc.vector.tensor_scalar_add(
                    yA[dh * 64:(dh + 1) * 64, :, :, :, :],
                    yA[dh * 64:(dh + 1) * 64, :, :, :, :],
                    bnb[dh * 64:(dh + 1) * 64, :])

            # ---- S8: FFN
            f1 = P.tile([128, 2, 16, 4, 2, 32], BF16, tag="slot20")
            for dh in range(2):
                for ch in range(8):
                    ps = PS.tile([128, 512], F32, tag="mm")
                    nc.tensor.matmul(
                        ps[:],
                        lhsT=pw1s[dh * 64:(dh + 1) * 64, :],
                        rhs=yA[dh * 64:(dh + 1) * 64, ch * 2:(ch + 1) * 2,
                               :, :, :],
                        start=True, stop=True)
                    nc.scalar.activation(
                        out=f1[:, dh, ch * 2:(ch + 1) * 2, :, :, :],
                        in_=ps[:], func=AF.Relu, bias=pw1bv[:], scale=pw1sv[:])
            f2t = P.tile([128, 16, 4, 2, 32], F32, tag="slot21")
            for dh in range(2):
                for ch in range(8):
                    ps = PS.tile([64, 512], F32, tag="mm")
                    nc.tensor.matmul(
                        ps[:], lhsT=pw2s[:],
                        rhs=f1[:, dh, ch * 2:(ch + 1) * 2, :, :, :],
                        start=True, stop=True)
                    nc.scalar.activation(
                        out=f2t[dh * 64:(dh + 1) * 64, ch * 2:(ch + 1) * 2,
                                :, :, :],
                        in_=ps[:], func=AF.Identity, bias=pw2bv[:],
                        scale=pw2sv[:])

            # ---- S9: out = y + f, write back
            outT = P.tile([128, 16, 4, 2, 32], F32, tag="slot4")
            nc.vector.tensor_tensor(out=outT[:], in0=yA[:], in1=f2t[:],
                                    op=ALU.add)
            for dh in range(2):
                for d in range(4):
                    nc.sync.dma_start(
                        out=yout_d[:, dh * 4 + d, :, :],
                        in_=outT[dh * 64:(dh + 1) * 64, :, d, :, :])

    _split_waits(nc)
    return nc


# ---------------------------------------------------------------------------
# host side
# ---------------------------------------------------------------------------

def _prep_inputs(inputs):
    import ml_dtypes

    x = np.asarray(inputs["x"], np.float32)
    w1 = np.asarray(inputs["w1"], np.float32)
    s1 = np.asarray(inputs["s1"], np.float32)
    b1 = np.asarray(inputs["b1"], np.float32)
    w2 = np.asarray(inputs["w2"], np.float32)
    s2 = np.asarray(inputs["s2"], np.float32)
    b2 = np.asarray(inputs["b2"], np.float32)
    w3 = np.asarray(inputs["w3"], np.float32)
    s3 = np.asarray(inputs["s3"], np.float32)
    b3 = np.asarray(inputs["b3"], np.float32)
    w4 = np.asarray(inputs["w4"], np.float32)
    b4 = np.asarray(inputs["b4"], np.float32)
    gn_g = np.asarray(inputs["gn_g"], np.float32)
    gn_b = np.asarray(inputs["gn_b"], np.float32)
    bn_s = np.asarray(inputs["bn_s"], np.float32)
    bn_b = np.asarray(inputs["bn_b"], np.float32)
    pw1_w = np.asarray(inputs["pw1_w"], np.float32)
    pw1_s = np.asarray(inputs["pw1_s"], np.float32)
    pw1_b = np.asarray(inputs["pw1_b"], np.float32)
    pw2_w = np.asarray(inputs["pw2_w"], np.float32)
    pw2_s = np.asarray(inputs["pw2_s"], np.float32)
    pw2_b = np.asarray(inputs["pw2_b"], np.float32)

    bf16 = ml_dtypes.bfloat16

    # shared weight tensors
    shared = {}
    shared["w1s"] = np.ascontiguousarray(np.tile(w1.T, (2, 1)))     # [128, 32]
    shared["s1v"] = s1.reshape(32, 1)
    shared["b1v"] = b1.reshape(32, 1)
    base = (w3 * (s2[None, :])).T                                   # [c, o]
    full = np.einsum('co,cxyz->xyzco', base, w2[:, 0])              # [5,5,5,c,o]
    mst = np.zeros((25, 128, 32), np.float32)
    mst2 = np.zeros((25, 32, 32), np.float32)
    for td in range(5):
        for th in range(5):
            t = td * 5 + th
            mst[t] = full[td, th, 0:4].reshape(128, 32)
            mst2[t] = full[td, th, 4]
    shared["mst"] = mst.transpose(1, 0, 2).astype(bf16)             # [128,25,32]
    shared["mst2"] = mst2.transpose(1, 0, 2).astype(bf16)           # [32,25,32]
    shared["s3v"] = s3.reshape(32, 1)
    shared["bias3"] = (s3 * (w3 @ b2) + b3).reshape(32, 1)
    w4T = np.ascontiguousarray(w4.T)                                # [32, 216]
    shared["w4lo"] = np.ascontiguousarray(np.tile(w4T[:, :108], (2, 1)))
    shared["w4hi"] = np.ascontiguousarray(np.tile(w4T[:, 108:], (2, 1)))
    shared["b4lo"] = b4[:108].reshape(108, 1)
    shared["b4hi"] = b4[108:].reshape(108, 1)
    shared["gglo"] = gn_g[:108].reshape(108, 1)
    shared["gghi"] = gn_g[108:].reshape(108, 1)
    shared["gblo"] = gn_b[:108].reshape(108, 1)
    shared["gbhi"] = gn_b[108:].reshape(108, 1)
    selm = np.zeros((108, 4), np.float32)
    for g in range(4):
        selm[g * 27:(g + 1) * 27, g] = 1.0
    shared["sel"] = selm
    repm = np.zeros((36, 108), np.float32)
    repm[0:4] = selm.T
    repm[32:36] = selm.T
    shared["rep"] = repm
    shared["bns"] = np.tile(bn_s, 2).reshape(128, 1)
    shared["bnb"] = np.tile(bn_b, 2).reshape(128, 1)
    shared["pw1s"] = np.ascontiguousarray(np.tile(pw1_w.T, (2, 1))) # [128, 128]
    shared["pw1sv"] = pw1_s.reshape(128, 1)
    shared["pw1bv"] = pw1_b.reshape(128, 1)
    shared["pw2s"] = np.ascontiguousarray(pw2_w.T).astype(bf16)     # [128, 64]
    shared["pw2sv"] = pw2_s.reshape(64, 1)
    shared["pw2bv"] = pw2_b.reshape(64, 1)

    in_maps = []
    for core in range(8):
        b, dq = core // 4, core % 4
        d0 = dq * 8
        m = dict(shared)

        # xa: [(2 dpair, 64 c), 6, 32, 32], d slices d0 + [-2..10), wrapped
        d_idx = (d0 + np.arange(-2, 10)) % D
        xd = x[b][:, d_idx]                            # [64, 12, 32, 32]
        xa = np.empty((128, 6, 32, 32), np.float32)
        xa[0:64] = xd[:, 0:6]
        xa[64:128] = xd[:, 6:12]
        m["xa"] = xa

        # xb: [(16 hb, 8 g), 8 c, 10 d', 4 h'', 34 w]
        d_idx2 = (d0 + np.arange(-1, 9)) % D
        xd2 = x[b][:, d_idx2]                          # [64, 10, 32, 32]
        xw = np.concatenate([xd2[..., -1:], xd2, xd2[..., :1]], axis=-1)
        h_idx = (2 * np.arange(16)[:, None] + np.arange(-1, 3)[None, :]) % H
        xh = xw[:, :, h_idx]                           # [64, 10, 16, 4, 34]
        arr = xh.reshape(8, 8, 10, 16, 4, 34)
        m["xb"] = np.ascontiguousarray(
            arr.transpose(3, 0, 1, 2, 4, 5).reshape(128, 8, 10, 4, 34)
        ).astype(bf16)

        m["dmL"] = np.full((128, 2), 0.0 if dq == 0 else 1.0, bf16)
        m["dmR"] = np.full((128, 2), 0.0 if dq == 3 else 1.0, bf16)
        in_maps.append(m)
    return in_maps


def _ensure_ntff_hook():
    import sys, types
    try:
        from antenv.axon_hooks import get_axon_ntff_profile_hook  # noqa
        return
    except ImportError:
        pass
    mod = types.ModuleType("antenv.axon_hooks")
    _h = [None]
    mod.set_axon_ntff_profile_hook = lambda h: _h.__setitem__(0, h)
    mod.get_axon_ntff_profile_hook = lambda: _h[0]
    sys.modules["antenv.axon_hooks"] = mod
    import antenv
    antenv.axon_hooks = mod
    try:
        from trn_agent_boot.trn_boot import _ntff_profile_via_ctypes
        mod.set_axon_ntff_profile_hook(
            _ntff_profile_via_ctypes("/opt/axon/libaxon_pjrt.so"))
    except Exception:
        pass


def kernel(**inputs):
    import os
    from concourse.bass_utils import run_bass_kernel_spmd

    if "nc" not in _CACHE:
        _CACHE["nc"] = _build_program()
    nc = _CACHE["nc"]

    in_maps = _prep_inputs(inputs)
    trace = bool(os.environ.get("KERNEL_TRACE"))
    if trace:
        _ensure_ntff_hook()
    res = run_bass_kernel_spmd(nc, in_maps, list(range(8)), trace=trace)
    globals()["LAST_EXEC_NS"] = res.exec_time_ns
    if trace and res.profile_json is not None:
        globals()["LAST_PROFILE"] = res.profile_json

    out = np.empty((B, C, D, H, W), np.float32)
    for core in range(8):
        b, dq = core // 4, core % 4
        out[b, :, dq * 8:(dq + 1) * 8] = res.results[core]["yout"]
    return out


# revision 18
# speedup vs baseline: 1.5011x; 1.4700x over previous
"""Trainium2 Bass kernel for nn_Block3D (LKP3D dynamic-kernel gen + SKA3D + FFN).

Sharding: 8 cores = batch(2) x D-quarters(4). Each core computes 8 D-slices
with host-prepared circular halos; GroupNorm stats are AllReduce'd across the
4 cores sharing a batch.

Per-core pipeline:
  S1  a1 = relu(bn1(w1 @ x))            PE f32r -> ACT evict into zero-padded,
                                        4x W-shift-replicated buffer (bf16)
  S2  a3 = relu(bn3(w3 @ bn2(dw5(a1)))) dw5 merged with w3*diag(s2) into 125
                                        dense 32x32 matmuls, K-packed 4 taps
  S3  wk = w4 @ a3 + b4; GroupNorm      stats via ACT accum_out + tiny
                                        AllReduce; GN folded to per-partition
                                        affine, applied in-place
  S4  wk -> (h-block, group)-partition layout via DMA transpose
  S5  SKA: 27 shifted DVE mult/adds (bf16), wk broadcast over the 8 channels
      of a group via a stride-0 free dim (no data movement)
  S6+ BN + residual (fp32), FFN (pw1/pw2), final residual, output
"""

import numpy as np

B, C, D, H, W = 2, 64, 32, 32, 32
G, K3 = 8, 27
GN_EPS = 1e-5

_CACHE = {}


# ---------------------------------------------------------------------------
# workarounds for this walrus build (max one sem wait per TPB instruction)
# ---------------------------------------------------------------------------

def _apply_tile_patches():
    from concourse import tile as _tile_mod
    from concourse.vector_clock import ScopedClock, VectorClock

    if getattr(_tile_mod.TileContext, "_drain_patched", False):
        return

    def _patched_drain_and_barrier(self, tick_clock, wait_clock):
        nc = self.nc
        gc = tick_clock.global_clock
        n = len(gc)
        vals = list(gc)
        for i in range(n):
            if vals[i] > 0:
                vec = [0] * n
                vec[i] = vals[i]
                nop_inst = nc.sync.nop(nofuse=True)
                wait_clock.add_sem_waits(
                    nop_inst.ins, ScopedClock({None: VectorClock(vec)})
                )
        nc.sync.drain()
        nc.all_engine_barrier()
        assert self.sems is not None
        popped = nc._tile_sem_poison_stack.pop()
        assert popped is self._sem_poison
        nc.clear_and_free_semaphores(list(self.sems.allocated().values()))
        nc.all_engine_barrier()

    _tile_mod.TileContext._drain_and_barrier = _patched_drain_and_barrier
    _tile_mod.TileContext._drain_patched = True


_WSPLIT = [0]


def _split_waits(nc):
    import concourse.mybir as mybir

    for _name, bbb in list(nc.bb_map.items()):
        bb = bbb.bb if hasattr(bbb, "bb") else bbb
        insts = bb.instructions
        new = []
        changed = False
        for inst in insts:
            si = inst.sync_info
            if si is not None and si.on_wait is not None and len(si.on_wait) > 1:
                waits = list(si.on_wait)
                for w in waits[:-1]:
                    _WSPLIT[0] += 1
                    new.append(
                        mybir.InstNoOp(
                            name=f"wsplit-{_WSPLIT[0]}",
                            engine=inst.engine,
                            sync_info=mybir.SyncInfo(on_wait=[w], on_update=[]),
                        )
                    )
                si.on_wait = [waits[-1]]
                inst.sync_info = si
                changed = True
            new.append(inst)
        if changed:
            bb.instructions[:] = new


# ---------------------------------------------------------------------------
# device program
# ---------------------------------------------------------------------------

def _build_program():
    import concourse.mybir as mybir
    from concourse import bass
    from concourse.tile import TileContext

    _apply_tile_patches()

    F32 = mybir.dt.float32
    F32R = mybir.dt.float32r
    BF16 = mybir.dt.bfloat16
    ALU = mybir.AluOpType
    AF = mybir.ActivationFunctionType

    nc = bass.Bass()

    def din(name, shape, dt=F32):
        return nc.declare_dram_parameter(name, list(shape), dt, isOutput=False)

    xa_d = din("xa", [128, 6, 32, 32], F32R)             # (dpair, c) x (6d, h, w)
    xb_d = din("xb", [128, 8, 10, 4, 34], BF16)    # (hb, g) x (c, d', h'', w)
    dmL_d = din("dmL", [128, 2], BF16)
    dmR_d = din("dmR", [128, 2], BF16)
    w1s_d = din("w1s", [128, 32], F32R)
    s1v_d = din("s1v", [32, 1])
    b1v_d = din("b1v", [32, 1])
    mst_d = din("mst", [128, 25, 32], BF16)
    mst2_d = din("mst2", [128, 25, 32], BF16)
    s3v_d = din("s3v", [32, 1])
    bias3_d = din("bias3", [32, 1])
    w4lo_d = din("w4lo", [64, 108], F32R)
    w4hi_d = din("w4hi", [64, 108], F32R)
    b4lo_d = din("b4lo", [108, 1])
    b4hi_d = din("b4hi", [108, 1])
    gglo_d = din("gglo", [108, 1])
    gghi_d = din("gghi", [108, 1])
    gblo_d = din("gblo", [108, 1])
    gbhi_d = din("gbhi", [108, 1])
    sel_d = din("sel", [108, 4])
    rep_d = din("rep", [36, 108])
    bns_d = din("bns", [128, 1])
    bnb_d = din("bnb", [128, 1])
    pw1s_d = din("pw1s", [128, 128], F32R)
    pw1sv_d = din("pw1sv", [128, 1])
    pw1bv_d = din("pw1bv", [128, 1])
    pw2s_d = din("pw2s", [128, 64], BF16)
    pw2sv_d = din("pw2sv", [64, 1])
    pw2bv_d = din("pw2bv", [64, 1])

    yout_d = nc.declare_dram_parameter("yout", [64, 8, 32, 32], F32, isOutput=True)

    NTOT = float(K3 * D * H * W)
    OFFS = [(di, hi, wi) for di in (-1, 0, 1) for hi in (-1, 0, 1)
            for wi in (-1, 0, 1)]

    with TileContext(nc) as tc:
        with tc.tile_pool(name="sb", bufs=1) as P, \
             tc.tile_pool(name="ps", bufs=2, space="PSUM") as PS, \
             tc.tile_pool(name="dram", bufs=1, space="DRAM") as PD:

            # persistent inputs / weights
            xb = P.tile([128, 8, 10, 4, 34], BF16, tag="xb")
            xa = P.tile([128, 6, 32, 32], F32R, tag="xa")
            w1s = P.tile([128, 32], F32R, tag="w1s")
            s1v = P.tile([32, 1], F32, tag="s1v")
            b1v = P.tile([32, 1], F32, tag="b1v")
            dmL = P.tile([128, 2], BF16, tag="dmL")
            dmR = P.tile([128, 2], BF16, tag="dmR")
            mst = P.tile([128, 25, 32], BF16, tag="mst")
            mst2 = P.tile([128, 25, 32], BF16, tag="mst2")
            s3v = P.tile([32, 1], F32, tag="s3v")
            bias3 = P.tile([32, 1], F32, tag="bias3")
            w4h = [P.tile([64, 108], F32R, tag=f"w4{h}", name=f"w4{h}") for h in range(2)]
            b4 = [P.tile([108, 1], F32, tag=f"b4{h}", name=f"b4{h}") for h in range(2)]
            gg = [P.tile([108, 1], F32, tag=f"gg{h}", name=f"gg{h}") for h in range(2)]
            gb = [P.tile([108, 1], F32, tag=f"gb{h}", name=f"gb{h}") for h in range(2)]
            sel = P.tile([108, 4], F32, tag="sel")
            rep = P.tile([36, 108], F32, tag="rep")
            bns = P.tile([128, 1], F32, tag="bns")
            bnb = P.tile([128, 1], F32, tag="bnb")
            pw1s = P.tile([128, 128], F32R, tag="pw1s")
            pw1sv = P.tile([128, 1], F32, tag="pw1sv")
            pw1bv = P.tile([128, 1], F32, tag="pw1bv")
            pw2s = P.tile([128, 64], BF16, tag="pw2s")
            pw2sv = P.tile([64, 1], F32, tag="pw2sv")
            pw2bv = P.tile([64, 1], F32, tag="pw2bv")

            for t, d in [(xb, xb_d), (xa, xa_d), (w1s, w1s_d), (s1v, s1v_d),
                         (b1v, b1v_d), (dmL, dmL_d), (dmR, dmR_d),
                         (mst, mst_d), (mst2, mst2_d), (s3v, s3v_d),
                         (bias3, bias3_d), (w4h[0], w4lo_d), (w4h[1], w4hi_d),
                         (b4[0], b4lo_d), (b4[1], b4hi_d),
                         (gg[0], gglo_d), (gg[1], gghi_d),
                         (gb[0], gblo_d), (gb[1], gbhi_d),
                         (sel, sel_d), (rep, rep_d), (bns, bns_d), (bnb, bnb_d),
                         (pw1s, pw1s_d), (pw1sv, pw1sv_d), (pw1bv, pw1bv_d),
                         (pw2s, pw2s_d), (pw2sv, pw2sv_d), (pw2bv, pw2bv_d)]:
                nc.sync.dma_start(out=t[:], in_=d[:])

            # ---- S1: a1 into a1rep block 0 (zero-padded, bf16)
            a1rep = P.tile([128, 12, 36, 36], BF16, tag="slot1")
            nc.gpsimd.memset(a1rep[:], 0.0)
            for p in range(2):
                for i in range(12):
                    dd = p * 6 + i // 2
                    hh = i % 2
                    ps = PS.tile([32, 16, 32], F32, tag="mm")
                    nc.tensor.matmul(
                        ps[:],
                        lhsT=w1s[p * 64:(p + 1) * 64, :],
                        rhs=xa[p * 64:(p + 1) * 64, i // 2,
                               hh * 16:(hh + 1) * 16, :],
                        start=True, stop=True)
                    nc.scalar.activation(
                        out=a1rep[0:32, dd, 2 + hh * 16:2 + (hh + 1) * 16, 2:34],
                        in_=ps[:], func=AF.Relu, bias=b1v[:], scale=s1v[:])
            # zero a1 halo slices that fall outside the global D range
            nc.vector.tensor_tensor(
                out=a1rep[0:32, 0:2, :, :], in0=a1rep[0:32, 0:2, :, :],
                in1=dmL[0:32, :, None, None].to_broadcast((32, 2, 36, 36)),
                op=ALU.mult)
            nc.vector.tensor_tensor(
                out=a1rep[0:32, 10:12, :, :], in0=a1rep[0:32, 10:12, :, :],
                in1=dmR[0:32, :, None, None].to_broadcast((32, 2, 36, 36)),
                op=ALU.mult)
            for j in range(1, 4):
                nc.sync.dma_start(
                    out=a1rep[j * 32:(j + 1) * 32, :, :, 0:36 - j],
                    in_=a1rep[0:32, :, :, j:36])

            # ---- S2: merged dw5 + w3
            a3t = [P.tile([64, 2, 32, 32], F32R, tag=f"slot3{i}", name=f"a3{i}") for i in range(2)]
            for d in range(8):
                for hh in range(2):
                    ps = PS.tile([32, 16, 32], F32, tag="mm")
                    for td in range(5):
                        for th in range(5):
                            t = td * 5 + th
                            hs = hh * 16 + th
                            nc.tensor.matmul(
                                ps[:], lhsT=mst[:, t, :],
                                rhs=a1rep[:, d + td, hs:hs + 16, 0:32],
                                start=(t == 0), stop=False)
                            nc.tensor.matmul(
                                ps[:], lhsT=mst2[:, t, :],
                                rhs=a1rep[:, d + td, hs:hs + 16, 4:36],
                                start=False, stop=(t == 24))
                    q = d // 2
                    nc.scalar.activation(
                        out=a3t[q // 2][(q % 2) * 32:(q % 2) * 32 + 32, d % 2,
                                        hh * 16:(hh + 1) * 16, :],
                        in_=ps[:], func=AF.Relu, bias=bias3[:], scale=s3v[:])

            # ---- S3: wk = w4 @ a3 + b4 (bf16), stats via accum_out
            wkA = [P.tile([108, 8, 32, 32], BF16, tag=f"slot2{h}", name=f"wkA{h}") for h in range(2)]
            sums = [P.tile([108, 2, 16], F32, tag=f"sums{h}", name=f"sums{h}") for h in range(2)]
            scr = P.tile([108, 512], F32, tag="scr")
            for h in range(2):
                for q in range(4):
                    for c2 in range(4):
                        idx = q * 4 + c2
                        dd = q * 2 + c2 // 2
                        hh = c2 % 2
                        ps = PS.tile([108, 512], F32, tag="mm")
                        nc.tensor.matmul(
                            ps[:],
                            lhsT=w4h[h][(q % 2) * 32:(q % 2) * 32 + 32, :],
                            rhs=a3t[q // 2][(q % 2) * 32:(q % 2) * 32 + 32, c2 // 2,
                                            hh * 16:(hh + 1) * 16, :],
                            start=True, stop=True)
                        nc.scalar.activation(
                            out=wkA[h][0:108, dd, hh * 16:(hh + 1) * 16, :],
                            in_=ps[:], func=AF.Identity, bias=b4[h][:],
                            accum_out=sums[h][:, 0, idx:idx + 1])
                        nc.scalar.activation(
                            out=scr[:], in_=ps[:], func=AF.Square, bias=b4[h][:],
                            accum_out=sums[h][:, 1, idx:idx + 1])

            # stats reduce -> [8, 2] -> AllReduce over the 4 same-batch cores
            s2t = [P.tile([108, 2], F32, tag=f"s2t{h}", name=f"s2t{h}") for h in range(2)]
            gstats = P.tile([36, 2], F32, tag="gstats")
            nc.vector.memset(gstats[:], 0.0)
            for h in range(2):
                nc.vector.tensor_reduce(
                    out=s2t[h][:], in_=sums[h][:], axis=mybir.AxisListType.X,
                    op=ALU.add)
                ps = PS.tile([4, 2], F32, tag="mmt")
                nc.tensor.matmul(ps[:], lhsT=sel[:], rhs=s2t[h][:],
                                 start=True, stop=True)
                nc.scalar.activation(out=gstats[h * 32:h * 32 + 4, :], in_=ps[:],
                                     func=AF.Copy)
            cin = PD.tile([36, 2], F32)
            cout = PD.tile([36, 2], F32)
            nc.sync.dma_start(out=cin[:], in_=gstats[:])
            nc.gpsimd.collective_compute(
                "AllReduce", ALU.add,
                replica_groups=[[0, 1, 2, 3], [4, 5, 6, 7]],
                ins=[cin[:].opt()], outs=[cout[:].opt()])
            gsum = P.tile([36, 2], F32, tag="gsum")
            nc.sync.dma_start(out=gsum[:], in_=cout[:])

            # mu, rsqrt(var+eps)
            m2 = P.tile([36, 2], F32, tag="m2")
            musq = P.tile([36, 1], F32, tag="musq")
            vs = P.tile([36, 1], F32, tag="vs")
            rv = P.tile([36, 1], F32, tag="rv")
            rs = P.tile([36, 1], F32, tag="rs")
            nc.scalar.mul(m2[:], gsum[:], 1.0 / NTOT)
            nc.scalar.activation(out=musq[:], in_=m2[:, 0:1], func=AF.Square)
            nc.vector.tensor_tensor(out=vs[:], in0=m2[:, 1:2], in1=musq[:],
                                    op=ALU.subtract)
            nc.vector.tensor_scalar_add(vs[:], vs[:], GN_EPS)
            nc.vector.reciprocal(rv[:], vs[:])
            nc.scalar.activation(out=rs[:], in_=rv[:], func=AF.Sqrt)

            # per-partition GN affine: alpha = r*gamma, beta = gn_b - mu*alpha
            alphas, betas = [], []
            for h in range(2):
                psr = PS.tile([108, 1], F32, tag="mmt")
                nc.tensor.matmul(psr[:], lhsT=rep[h * 32:h * 32 + 4, :], rhs=rs[h * 32:h * 32 + 4, :],
                                 start=True, stop=True)
                rb = P.tile([108, 1], F32, tag=f"rb{h}")
                nc.scalar.activation(out=rb[:], in_=psr[:], func=AF.Copy)
                psm = PS.tile([108, 1], F32, tag="mmt")
                nc.tensor.matmul(psm[:], lhsT=rep[h * 32:h * 32 + 4, :],
                                 rhs=m2[h * 32:h * 32 + 4, 0:1],
                                 start=True, stop=True)
                mb = P.tile([108, 1], F32, tag=f"mb{h}")
                nc.scalar.activation(out=mb[:], in_=psm[:], func=AF.Copy)
                alpha = P.tile([108, 1], F32, tag=f"al{h}")
                beta = P.tile([108, 1], F32, tag=f"be{h}")
                nc.vector.tensor_tensor(out=alpha[:], in0=rb[:], in1=gg[h][:],
                                        op=ALU.mult)
                nc.vector.tensor_tensor(out=beta[:], in0=mb[:], in1=alpha[:],
                                        op=ALU.mult)
                nc.vector.tensor_tensor(out=beta[:], in0=gb[h][:], in1=beta[:],
                                        op=ALU.subtract)
                alphab = P.tile([108, 1], BF16, tag=f"alb{h}", name=f"alb{h}")
                betab = P.tile([108, 1], BF16, tag=f"beb{h}", name=f"beb{h}")
                nc.vector.tensor_copy(alphab[:], alpha[:])
                nc.vector.tensor_copy(betab[:], beta[:])
                alphas.append(alphab)
                betas.append(betab)

            # ---- S4: transpose RAW wk into (hb, g) partition layout (overlaps
            # with the GN-stats collective), then apply the GN affine in B.
            wkB = P.tile([128, 27, 8, 2, 32], BF16, tag="slot1")
            _engs = [nc.sync, nc.scalar, nc.gpsimd]
            for hb in range(16):
                for h in range(2):
                    _engs[(hb * 2 + h) % 3].dma_start(
                        out=wkB[hb * 8 + h * 4:hb * 8 + h * 4 + 4, :, :, :, :],
                        in_=wkA[h][0:108, :, 2 * hb:2 * hb + 2, :])
            # distribute alpha/beta to (hb, g) partitions: ab8 rows (g) x (k | k)
            ab8 = P.tile([8, 54], BF16, tag="ab8")
            for h in range(2):
                nc.sync.dma_start(out=ab8[h * 4:(h + 1) * 4, 0:27],
                                  in_=alphas[h][:])
                nc.sync.dma_start(out=ab8[h * 4:(h + 1) * 4, 27:54],
                                  in_=betas[h][:])
            abB = P.tile([128, 54], BF16, tag="abB")
            for hb in range(16):
                nc.sync.dma_start(out=abB[hb * 8:(hb + 1) * 8, :], in_=ab8[:])
            nc.vector.tensor_tensor(
                out=wkB[:], in0=wkB[:],
                in1=abB[:, 0:27, None].to_broadcast((128, 27, 512)),
                op=ALU.mult)
            nc.vector.tensor_tensor(
                out=wkB[:], in0=wkB[:],
                in1=abB[:, 27:54, None].to_broadcast((128, 27, 512)),
                op=ALU.add)

            # ---- S5: SKA
            accB = P.tile([128, 8, 8, 2, 32], BF16, tag="slot4")
            prodB = P.tile([128, 8, 8, 2, 32], BF16, tag="slot5")
            for k, (di, hi, wi) in enumerate(OFFS):
                for hp in range(2):
                    xsl = xb[:, :, 1 + di:9 + di, 1 + hi + hp, 1 + wi:33 + wi]
                    wsl = wkB[:, k:k + 1, :, hp, :].to_broadcast((128, 8, 8, 32))
                    if k == 0:
                        nc.vector.tensor_tensor(out=accB[:, :, :, hp, :],
                                                in0=xsl, in1=wsl, op=ALU.mult)
                    else:
                        nc.vector.tensor_tensor(out=prodB[:, :, :, hp, :],
                                                in0=xsl, in1=wsl, op=ALU.mult)
                        nc.vector.tensor_tensor(out=accB[:, :, :, hp, :],
                                                in0=accB[:, :, :, hp, :],
                                                in1=prodB[:, :, :, hp, :],
                                                op=ALU.add)

            # convert to f32 for the transpose back
            accF = P.tile([128, 8, 8, 2, 32], F32, tag="slot5")
            nc.vector.tensor_copy(accF[:], accB[:])

            # ---- S7: transpose sk back to channel-partition layout
            skA = P.tile([128, 16, 4, 2, 32], F32, tag="slot30")
            for hb in range(16):
                for dh in range(2):
                    _engs[(hb * 2 + dh) % 3].dma_start(
                        out=skA[dh * 64:(dh + 1) * 64, hb, :, :, :],
                        in_=accF[hb * 8:(hb + 1) * 8, :, dh * 4:(dh + 1) * 4, :, :])

            # ---- S6': y = bn_s*sk + bn_b + x   (fp32)
            yA = P.tile([128, 16, 4, 2, 32], F32, tag="yA")
            for dh in range(2):
                xs_all = (xa[0:64, 2:6, :, :] if dh == 0 else xa[64:128, 0:4, :, :]).bitcast(F32)
                for hb in range(16):
                    nc.vector.scalar_tensor_tensor(
                        out=yA[dh * 64:(dh + 1) * 64, hb, :, :, :],
                        in0=skA[dh * 64:(dh + 1) * 64, hb, :, :, :],
                        scalar=bns[dh * 64:(dh + 1) * 64, :],
                        in1=xs_all[:, :, 2 * hb:2 * hb + 2, :],
                        op0=ALU.mult, op1=ALU.add)
                nc.vector.tensor_scalar_add(
                    yA[dh * 64:(dh + 1) * 64, :, :, :, :],
                    yA[dh * 64:(dh + 1) * 64, :, :, :, :],
                    bnb[dh * 64:(dh + 1) * 64, :])

            # ---- S8: FFN
            yAr = P.tile([128, 16, 4, 2, 32], F32R, tag="slot4", name="yAr")
            nc.scalar.activation(out=yAr[:], in_=yA[:].bitcast(F32R), func=AF.Copy)
            f1 = P.tile([128, 2, 16, 4, 2, 32], BF16, tag="slot20")
            for dh in range(2):
                for ch in range(8):
                    ps = PS.tile([128, 512], F32, tag="mm")
                    nc.tensor.matmul(
                        ps[:],
                        lhsT=pw1s[dh * 64:(dh + 1) * 64, :],
                        rhs=yAr[dh * 64:(dh + 1) * 64, ch * 2:(ch + 1) * 2,
                                :, :, :],
                        start=True, stop=True)
                    nc.scalar.activation(
                        out=f1[:, dh, ch * 2:(ch + 1) * 2, :, :, :],
                        in_=ps[:], func=AF.Relu, bias=pw1bv[:], scale=pw1sv[:])
            f2t = P.tile([128, 16, 4, 2, 32], F32, tag="slot21")
            for dh in range(2):
                for ch in range(8):
                    ps = PS.tile([64, 512], F32, tag="mm")
                    nc.tensor.matmul(
                        ps[:], lhsT=pw2s[:],
                        rhs=f1[:, dh, ch * 2:(ch + 1) * 2, :, :, :],
                        start=True, stop=True)
                    nc.scalar.activation(
                        out=f2t[dh * 64:(dh + 1) * 64, ch * 2:(ch + 1) * 2,
                                :, :, :],
                        in_=ps[:], func=AF.Identity, bias=pw2bv[:],
                        scale=pw2sv[:])

            # ---- S9: out = y + f, write back
            outT = P.tile([128, 16, 4, 2, 32], F32, tag="slot4")
            nc.vector.tensor_tensor(out=outT[:], in0=yA[:], in1=f2t[:],
                                    op=ALU.add)
            for dh in range(2):
                for d in range(4):
                    nc.sync.dma_start(
                        out=yout_d[:, dh * 4 + d, :, :],
                        in_=outT[dh * 64:(dh + 1) * 64, :, d, :, :])

    _split_waits(nc)
    return nc


# ---------------------------------------------------------------------------
# host side
# ---------------------------------------------------------------------------

def _prep_inputs(inputs):
    import ml_dtypes

    x = np.asarray(inputs["x"], np.float32)
    w1 = np.asarray(inputs["w1"], np.float32)
    s1 = np.asarray(inputs["s1"], np.float32)
    b1 = np.asarray(inputs["b1"], np.float32)
    w2 = np.asarray(inputs["w2"], np.float32)
    s2 = np.asarray(inputs["s2"], np.float32)
    b2 = np.asarray(inputs["b2"], np.float32)
    w3 = np.asarray(inputs["w3"], np.float32)
    s3 = np.asarray(inputs["s3"], np.float32)
    b3 = np.asarray(inputs["b3"], np.float32)
    w4 = np.asarray(inputs["w4"], np.float32)
    b4 = np.asarray(inputs["b4"], np.float32)
    gn_g = np.asarray(inputs["gn_g"], np.float32)
    gn_b = np.asarray(inputs["gn_b"], np.float32)
    bn_s = np.asarray(inputs["bn_s"], np.float32)
    bn_b = np.asarray(inputs["bn_b"], np.float32)
    pw1_w = np.asarray(inputs["pw1_w"], np.float32)
    pw1_s = np.asarray(inputs["pw1_s"], np.float32)
    pw1_b = np.asarray(inputs["pw1_b"], np.float32)
    pw2_w = np.asarray(inputs["pw2_w"], np.float32)
    pw2_s = np.asarray(inputs["pw2_s"], np.float32)
    pw2_b = np.asarray(inputs["pw2_b"], np.float32)

    bf16 = ml_dtypes.bfloat16

    # shared weight tensors
    shared = {}
    shared["w1s"] = np.ascontiguousarray(np.tile(w1.T, (2, 1)))     # [128, 32]
    shared["s1v"] = s1.reshape(32, 1)
    shared["b1v"] = b1.reshape(32, 1)
    base = (w3 * (s2[None, :])).T                                   # [c, o]
    full = np.einsum('co,cxyz->xyzco', base, w2[:, 0])              # [5,5,5,c,o]
    mst = np.zeros((25, 128, 32), np.float32)
    mst2 = np.zeros((25, 32, 32), np.float32)
    for td in range(5):
        for th in range(5):
            t = td * 5 + th
            mst[t] = full[td, th, 0:4].reshape(128, 32)
            mst2[t] = full[td, th, 4]
    shared["mst"] = mst.transpose(1, 0, 2).astype(bf16)             # [128,25,32]
    mst2z = np.zeros((25, 128, 32), np.float32)
    mst2z[:, 0:32, :] = mst2
    shared["mst2"] = mst2z.transpose(1, 0, 2).astype(bf16)          # [128,25,32]
    shared["s3v"] = s3.reshape(32, 1)
    shared["bias3"] = (s3 * (w3 @ b2) + b3).reshape(32, 1)
    w4T = np.ascontiguousarray(w4.T)                                # [32, 216]
    shared["w4lo"] = np.ascontiguousarray(np.tile(w4T[:, :108], (2, 1)))
    shared["w4hi"] = np.ascontiguousarray(np.tile(w4T[:, 108:], (2, 1)))
    shared["b4lo"] = b4[:108].reshape(108, 1)
    shared["b4hi"] = b4[108:].reshape(108, 1)
    shared["gglo"] = gn_g[:108].reshape(108, 1)
    shared["gghi"] = gn_g[108:].reshape(108, 1)
    shared["gblo"] = gn_b[:108].reshape(108, 1)
    shared["gbhi"] = gn_b[108:].reshape(108, 1)
    selm = np.zeros((108, 4), np.float32)
    for g in range(4):
        selm[g * 27:(g + 1) * 27, g] = 1.0
    shared["sel"] = selm
    repm = np.zeros((36, 108), np.float32)
    repm[0:4] = selm.T
    repm[32:36] = selm.T
    shared["rep"] = repm
    shared["bns"] = np.tile(bn_s, 2).reshape(128, 1)
    shared["bnb"] = np.tile(bn_b, 2).reshape(128, 1)
    shared["pw1s"] = np.ascontiguousarray(np.tile(pw1_w.T, (2, 1))) # [128, 128]
    shared["pw1sv"] = pw1_s.reshape(128, 1)
    shared["pw1bv"] = pw1_b.reshape(128, 1)
    shared["pw2s"] = np.ascontiguousarray(pw2_w.T).astype(bf16)     # [128, 64]
    shared["pw2sv"] = pw2_s.reshape(64, 1)
    shared["pw2bv"] = pw2_b.reshape(64, 1)

    in_maps = []
    for core in range(8):
        b, dq = core // 4, core % 4
        d0 = dq * 8
        m = dict(shared)

        # xa: [(2 dpair, 64 c), 6, 32, 32], d slices d0 + [-2..10), wrapped
        d_idx = (d0 + np.arange(-2, 10)) % D
        xd = x[b][:, d_idx]                            # [64, 12, 32, 32]
        xa = np.empty((128, 6, 32, 32), np.float32)
        xa[0:64] = xd[:, 0:6]
        xa[64:128] = xd[:, 6:12]
        m["xa"] = xa

        # xb: [(16 hb, 8 g), 8 c, 10 d', 4 h'', 34 w]
        d_idx2 = (d0 + np.arange(-1, 9)) % D
        xd2 = x[b][:, d_idx2]                          # [64, 10, 32, 32]
        xw = np.concatenate([xd2[..., -1:], xd2, xd2[..., :1]], axis=-1)
        h_idx = (2 * np.arange(16)[:, None] + np.arange(-1, 3)[None, :]) % H
        xh = xw[:, :, h_idx]                           # [64, 10, 16, 4, 34]
        arr = xh.reshape(8, 8, 10, 16, 4, 34)
        m["xb"] = np.ascontiguousarray(
            arr.transpose(3, 0, 1, 2, 4, 5).reshape(128, 8, 10, 4, 34)
        ).astype(bf16)

        m["dmL"] = np.full((128, 2), 0.0 if dq == 0 else 1.0, bf16)
        m["dmR"] = np.full((128, 2), 0.0 if dq == 3 else 1.0, bf16)
        in_maps.append(m)
    return in_maps


def _ensure_ntff_hook():
    import sys, types
    try:
        from antenv.axon_hooks import get_axon_ntff_profile_hook  # noqa
        return
    except ImportError:
        pass
    mod = types.ModuleType("antenv.axon_hooks")
    _h = [None]
    mod.set_axon_ntff_profile_hook = lambda h: _h.__setitem__(0, h)
    mod.get_axon_ntff_profile_hook = lambda: _h[0]
    sys.modules["antenv.axon_hooks"] = mod
    import antenv
    antenv.axon_hooks = mod
    try:
        from trn_agent_boot.trn_boot import _ntff_profile_via_ctypes
        mod.set_axon_ntff_profile_hook(
            _ntff_profile_via_ctypes("/opt/axon/libaxon_pjrt.so"))
    except Exception:
        pass


def kernel(**inputs):
    import os
    from concourse.bass_utils import run_bass_kernel_spmd

    if "nc" not in _CACHE:
        _CACHE["nc"] = _build_program()
    nc = _CACHE["nc"]

    in_maps = _prep_inputs(inputs)
    trace = bool(os.environ.get("KERNEL_TRACE"))
    if trace:
        _ensure_ntff_hook()
    res = run_bass_kernel_spmd(nc, in_maps, list(range(8)), trace=trace)
    globals()["LAST_EXEC_NS"] = res.exec_time_ns
    if trace and res.profile_json is not None:
        globals()["LAST_PROFILE"] = res.profile_json

    out = np.empty((B, C, D, H, W), np.float32)
    for core in range(8):
        b, dq = core // 4, core % 4
        out[b, :, dq * 8:(dq + 1) * 8] = res.results[core]["yout"]
    return out
